# revision 19
# baseline (speedup 1.0000x reference)
"""Trainium2 Bass kernel for nn_Attention_21285857919576.

Strategy: 8 cores = 4 batches x 2 head-groups (tensor parallel over heads).
Each core computes, for its (batch b, head-group g):
  - Q/K/V projections (s-major) on TensorE in bf16 (lhsT = x^T tiles)
  - per-head sumsq on DVE; rsqrt via ACT exp(-0.5*ln(ms)) (single table set);
    RoPE applied to the UNNORMALIZED q/k (rotation commutes with the
    per-head rms scalar) on DVE + GpSimd
  - transpose+normalize q,k into d-major slabs (QT/KT) via PE matmuls
    against diag(rms_inv) tiles, software-pipelined one s-tile late
  - causal attention in transposed orientation (no transposes of p needed):
      scoresT[k,q] = KT_tile.T @ QT   (one matmul per k-tile, no accum)
      exp on ACT (no max subtraction: RMSNorm+RoPE bound |scores|; 1/sqrt(D)
      and the norm weights are folded into cos/sin host-side)
      p_acc += p on DVE (bf16 2x mode) -- the softmax denominator leaves
      the PE: ONE ones[128,128].T @ p_acc matmul per (head, q-chunk)
      replaces the per-k-tile den matmul stream (saves ~50us of PE issue)
      outT[d,q]  += V_tile.T @ p      (V is s-major straight from projection;
      AV delayed two k-tiles so the exp chain never head-of-line blocks PE)
  - partial output = attnoutT.T @ woT, interleaved INTO the attention phase:
    the wo matmuls of q-chunk qc-1 are slotted between the attention k-tiles
    of q-chunk qc (fractional-credit pacing), so ACT exp hides under wo
    PE streams instead of pacing the PE
Host sums the two partial outputs per batch. No collectives; the 8 cores are
fully independent and perfectly load balanced.

All large matmuls run in bf16 (fp32 PSUM accumulate). Host-side preprocessing
folds q/k norm weights, the rotate-half sign, and the score scale into the
cos/sin tables, and pre-transposes x and the weights so every DMA is
layout-natural. Causal masking is a multiplicative 0/1 bf16 mask applied to p
AFTER exp (exact -- unmasked scores are bounded so exp is finite).
The first x-tile lives in a persistent SBUF tile loaded once at startup and
reused by BOTH projection sweeps (kills the sweep-boundary DMA stall).
PSUM: 4 rotating score banks + 2 attention-out banks + 2 wo banks.
"""
import sys
import numpy as np
import ml_dtypes

for _p in ("/opt/trn_rl_repo", "/opt/pypackages"):
    if _p not in sys.path:
        sys.path.append(_p)

import concourse.bass as bass
from concourse import bacc, mybir, tile
from concourse.bass_utils import run_bass_kernel_spmd


def _install_ntff_hook_shim():
    """The staged antenv package lacks axon_hooks; provide it so
    run_bass_kernel_spmd(trace=True) can drive NTFF profiling via the
    injected libaxon .so (same mechanism trn_boot would register)."""
    import types
    if "antenv.axon_hooks" in sys.modules:
        return
    mod = types.ModuleType("antenv.axon_hooks")
    _state = {"hook": None}
    mod.set_axon_ntff_profile_hook = lambda h: _state.__setitem__("hook", h)
    mod.get_axon_ntff_profile_hook = lambda: _state["hook"]
    sys.modules["antenv.axon_hooks"] = mod
    try:
        import antenv
        antenv.axon_hooks = mod
    except ImportError:
        pass
    try:
        from trn_agent_boot.trn_boot import _ntff_profile_via_ctypes
        import os
        so = "/opt/axon/libaxon_pjrt.so"
        if os.path.exists(so):
            mod.set_axon_ntff_profile_hook(_ntff_profile_via_ctypes(so))
    except Exception:
        pass


_install_ntff_hook_shim()


def _install_act_table_patch():
    """Force Exp/Ln/Copy/Square onto the single natural_log_exp_and_others
    ACT table set: blank every other set containing exp/ln so the
    table-load chooser can't alternate between sets (each switch costs
    ~2.7us and we interleave Ln (rms) with Exp (softmax))."""
    import concourse.hw_specs as hw_specs
    import concourse.bacc as bacc_mod
    if getattr(hw_specs, "_act_patch", False):
        return
    orig = hw_specs.get_activation_tables

    def patched(module_arch):
        tables = orig(module_arch)
        keep = "natural_log_exp_and_others"
        if keep in tables:
            for name, fns in tables.items():
                if name != keep and any(f.name in ("Exp", "Ln") for f in fns):
                    tables[name] = set()
        return tables

    hw_specs.get_activation_tables = patched
    if getattr(bacc_mod, "get_activation_tables", None) is orig:
        bacc_mod.get_activation_tables = patched
    hw_specs._act_patch = True


_install_act_table_patch()

BF = ml_dtypes.bfloat16
F32 = mybir.dt.float32
BF16 = mybir.dt.bfloat16
ALU = mybir.AluOpType
AF = mybir.ActivationFunctionType

S, HSD, D = 2048, 2048, 128
NQ, NKV = 8, 4          # per-core q heads / kv heads
NT = S // 128           # 16 s-tiles
EPS = 1e-6


def _emit(tc, aps):
    from collections import deque
    nc = tc.nc
    xT = aps["xT"]
    wqT = aps["wqT"]
    wkT = aps["wkT"]
    wvT = aps["wvT"]
    woT = aps["woT"]
    cosq = aps["cosq"]
    sinq = aps["sinq"]
    cosk = aps["cosk"]
    sink = aps["sink"]
    maskd = aps["mask"]
    outd = aps["out"]

    xT3 = xT.rearrange("(ho hi) s -> hi ho s", hi=128)       # [128, 16, 2048]
    wqT3 = wqT.rearrange("(ho hi) f -> hi ho f", hi=128)     # [128, 16, 1024]
    wkT3 = wkT.rearrange("(ho hi) f -> hi ho f", hi=128)     # [128, 16, 512]
    wvT3 = wvT.rearrange("(ho hi) f -> hi ho f", hi=128)
    woT3 = woT.rearrange("(fo fi) o -> fi fo o", fi=128)     # [128, 8, 2048]

    from contextlib import ExitStack
    with ExitStack() as ctx:
        singles = ctx.enter_context(tc.tile_pool(name="singles", bufs=1))
        wsl = ctx.enter_context(tc.tile_pool(name="wsl", bufs=2))
        xtp = ctx.enter_context(tc.tile_pool(name="xtp", bufs=3))
        trig = ctx.enter_context(tc.tile_pool(name="trig", bufs=4))
        qfp = ctx.enter_context(tc.tile_pool(name="qfp", bufs=4))
        t12 = ctx.enter_context(tc.tile_pool(name="t12", bufs=6))
        sqp = ctx.enter_context(tc.tile_pool(name="sqp", bufs=2))
        tiny = ctx.enter_context(tc.tile_pool(name="tiny", bufs=8))
        qsbp = ctx.enter_context(tc.tile_pool(name="qsbp", bufs=3))
        ksbp = ctx.enter_context(tc.tile_pool(name="ksbp", bufs=3))
        pp = ctx.enter_context(tc.tile_pool(name="pp", bufs=8))
        outp = ctx.enter_context(tc.tile_pool(name="outp", bufs=3))
        rdenp = ctx.enter_context(tc.tile_pool(name="rdenp", bufs=3))
        diagp = ctx.enter_context(tc.tile_pool(name="diagp", bufs=4))
        paccp = ctx.enter_context(tc.tile_pool(name="paccp", bufs=2))
        psSC = ctx.enter_context(tc.tile_pool(name="psSC", bufs=4, space="PSUM"))
        psoP = ctx.enter_context(tc.tile_pool(name="psoP", bufs=2,
                                              space="PSUM"))
        woPp = ctx.enter_context(tc.tile_pool(name="woPp", bufs=2,
                                              space="PSUM"))

        QT = singles.tile([128, NQ, S], BF16)    # [d, h, s]
        KT = singles.tile([128, NKV, S], BF16)   # [d, kv, s]
        Vs = singles.tile([128, NT, NKV * D], BF16)  # [s_i, s_o, f]
        AOT = singles.tile([128, NQ, S], BF16)   # [d, h, s]
        mask_sb = singles.tile([128, 128], BF16)
        ones_col = singles.tile([128, 128], BF16)

        eps_sb = singles.tile([128, 1], F32)
        wkH = singles.tile([128, 4, 512], BF16)
        ident = singles.tile([128, 128], BF16)
        xtF = singles.tile([128, 16, 128], BF16)   # first s-tile, both sweeps
        nc.vector.memset(ones_col[:], 1.0)
        nc.vector.memset(eps_sb[:], EPS)
        from concourse.masks import make_identity
        make_identity(nc, ident[:])

        def process_qk(ps, cos_t, sin_t, out_ap):
            """RoPE 4 heads from psum tile ps [128,4,128] (unnormalized),
            write bf16 to out_ap [128, 512]; return rms_inv [128, 4] tile
            (applied later via the diag transpose matmul)."""
            qf = qfp.tile([128, 4, 128], BF16, tag="qf")
            nc.scalar.copy(qf[:], ps[:])          # ACT: psum -> sbuf
            sq = sqp.tile([128, 4, 128], BF16, tag="sq")
            ss = tiny.tile([128, 4], F32, tag="ss")
            for hb in range(4):
                nc.vector.scalar_tensor_tensor(
                    out=sq[:, hb], in0=qf[:, hb], scalar=1.0, in1=qf[:, hb],
                    op0=ALU.mult, op1=ALU.mult, accum_out=ss[:, hb:hb + 1])
            lnt = tiny.tile([128, 4], F32, tag="lnt")
            nc.scalar.activation(lnt[:], ss[:], AF.Ln, scale=1.0 / D,
                                 bias=eps_sb[:])
            rmsi = tiny.tile([128, 4], BF16, tag="rmsi")
            nc.scalar.activation(rmsi[:], lnt[:], AF.Exp, scale=-0.5)
            cosb = cos_t[:].unsqueeze(1).to_broadcast([128, 4, 128])
            sinb = sin_t[:].unsqueeze(1).to_broadcast([128, 4, 128])
            t1 = t12.tile([128, 4, 128], BF16, tag="t1")
            t2 = t12.tile([128, 4, 128], BF16, tag="t2")
            nc.vector.tensor_tensor(out=t1[:], in0=qf[:], in1=cosb, op=ALU.mult)
            qfb = qf[:]
            rot = bass.AP(tensor=qfb.tensor, offset=qfb.offset + 64,
                          ap=[qfb.ap[0], [128, 4], [-64, 2], [1, 64]])
            t2v = t2[:].rearrange("p h (j d) -> p h j d", j=2)
            sinv = sinb.rearrange("p h (j d) -> p h j d", j=2)
            nc.vector.tensor_tensor(out=t2v, in0=rot, in1=sinv, op=ALU.mult)
            nc.gpsimd.tensor_tensor(out=out_ap, in0=t1[:], in1=t2[:], op=ALU.add)
            return rmsi

        def transpose4(src_ap, rmsi, dst_ap):
            """Transpose+normalize 4 head-tiles: src_ap [128 s, 512 f] bf16
            (roped, unnormalized), rmsi [128 s, 4], into dst_ap
            [128 d, 4, 128 s] (bf16 slab slice) via PE diag matmuls."""
            pst = psSC.tile([128, 4, 128], F32, tag="sc")
            diag4 = diagp.tile([128, 4, 128], BF16, tag="diag")
            identb = ident[:].unsqueeze(1).to_broadcast([128, 4, 128])
            rmsib = rmsi[:].unsqueeze(2).to_broadcast([128, 4, 128])
            nc.vector.tensor_tensor(out=diag4[:], in0=identb, in1=rmsib,
                                    op=ALU.mult)
            for hb in range(4):
                nc.tensor.matmul(pst[:, hb],
                                 lhsT=src_ap[:, hb * 128:(hb + 1) * 128],
                                 rhs=diag4[:, hb], start=True, stop=True)
            nc.scalar.copy(dst_ap, pst[:])

        def attn_head(qc, h, feed_credit):
            """QK -> exp -> p_acc (DVE) -> AV for one (q-chunk, head).
            AV delayed two k-tiles; den is a single matmul off p_acc.
            feed_credit() is called once per k-tile to slot wo-projection
            matmuls into the PE stream so ACT exp time stays hidden."""
            qbase = qc * 512
            kv = h // 2
            nkt = 4 * qc + 4
            pso = psoP.tile([128, 512], F32, tag="pso")
            pacc = paccp.tile([128, 512], BF16, tag="pacc")

            def emit_av(p, kt, lo):
                nc.tensor.matmul(
                    pso[:, lo:512],
                    lhsT=Vs[:, kt, kv * 128:(kv + 1) * 128],
                    rhs=p[:, lo:512],
                    start=(kt == 0), stop=(kt == nkt - 1))

            pend_av = deque()
            for kt in range(nkt):
                j = kt - 4 * qc
                lo = 128 * j if j >= 0 else 0
                psc = psSC.tile([128, 512], F32, tag="sc")
                nc.tensor.matmul(
                    psc[:, lo:512],
                    lhsT=KT[:, kv, kt * 128:(kt + 1) * 128],
                    rhs=QT[:, h, qbase + lo:qbase + 512],
                    start=True, stop=True)
                p = pp.tile([128, 512], BF16, tag="p")
                nc.scalar.activation(p[:, lo:512], psc[:, lo:512], AF.Exp)
                if j >= 0:
                    nc.vector.tensor_tensor(
                        out=p[:, lo:lo + 128], in0=p[:, lo:lo + 128],
                        in1=mask_sb[:], op=ALU.mult)
                if kt == 0:
                    nc.vector.tensor_copy(out=pacc[:], in_=p[:])
                else:
                    nc.vector.tensor_tensor(
                        out=pacc[:, lo:512], in0=pacc[:, lo:512],
                        in1=p[:, lo:512], op=ALU.add)
                pend_av.append((p, kt, lo))
                if len(pend_av) > 3:
                    emit_av(*pend_av.popleft())
                feed_credit()
            while pend_av:
                emit_av(*pend_av.popleft())
            return pso, pacc

        def head_epilogue(qc, h, pso, pacc):
            psden = psSC.tile([128, 512], F32, tag="sc")
            nc.tensor.matmul(psden[:], lhsT=ones_col[:], rhs=pacc[:],
                             start=True, stop=True)
            rden = rdenp.tile([128, 512], F32, tag="rden")
            nc.vector.reciprocal_approx_fast(out=rden[:], in_=psden[:])
            nc.vector.tensor_tensor(
                out=AOT[:, h, qc * 512:qc * 512 + 512], in0=pso[:],
                in1=rden[:], op=ALU.mult)

        def wo_feeder(units):
            """Generator emitting one wo matmul per next(); finishes each
            (s-tile, out-chunk) unit with the psum->sbuf copy + store."""
            for st, oc in units:
                ssl = slice(st * 128, (st + 1) * 128)
                osl = slice(oc * 512, (oc + 1) * 512)
                po = woPp.tile([128, 512], F32, tag="po")
                for fc in range(NQ):
                    w = woA if fc < 4 else woB
                    nc.tensor.matmul(po[:], lhsT=AOT[:, fc, ssl],
                                     rhs=w[:, fc % 4, osl],
                                     start=(fc == 0), stop=(fc == 7))
                    if fc == 7:
                        osb = outp.tile([128, 512], F32, tag="osb")
                        nc.vector.tensor_copy(out=osb[:], in_=po[:])
                        nc.sync.dma_start(out=outd[ssl, osl], in_=osb[:])
                    yield

        def attention_block(qc, units, delay_kts=0):
            """All 8 heads of q-chunk qc; the wo matmuls of q-chunk qc-1
            are slotted between attention k-tiles (via feed_credit) so
            the PE always has filler while ACT exp runs. delay_kts skips
            feeding for the first k-tiles (covers the wo-weight DMA)."""
            feeder = wo_feeder(units)
            nkts = NQ * (4 * qc + 4)
            per_kt = max(0.0, (len(units) * NQ - NQ) / max(1, nkts - delay_kts))
            state = {"c": 0.0, "kt": 0}

            def feed(n):
                for _ in range(n):
                    if next(feeder, None) is None:
                        return

            def feed_credit():
                state["kt"] += 1
                if state["kt"] <= delay_kts:
                    return
                state["c"] += per_kt
                while state["c"] >= 1.0:
                    feed(1)
                    state["c"] -= 1.0

            for h in range(NQ):
                pso, pacc = attn_head(qc, h, feed_credit)
                if state["kt"] > delay_kts:
                    feed(1)
                head_epilogue(qc, h, pso, pacc)
            for _ in feeder:
                pass

        # ---- sweep 1: Q projection + rms/rope + transpose into QT ----
        pend_q = []
        wqA = wsl.tile([128, 16, 512], BF16, tag="w")
        wqB = wsl.tile([128, 16, 512], BF16, tag="w")
        nc.scalar.dma_start(out=xtF[:, 0:8, :], in_=xT3[:, 0:8, 0:128])
        nc.sync.dma_start(out=wqA[:, 0:1, :], in_=wqT3[:, 0:1, 0:512])
        nc.sync.dma_start(out=wqB[:, 0:1, :], in_=wqT3[:, 0:1, 512:1024])
        nc.scalar.dma_start(out=xtF[:, 8:16, :], in_=xT3[:, 8:16, 0:128])
        nc.sync.dma_start(out=wqA[:, 1:4, :], in_=wqT3[:, 1:4, 0:512])
        nc.sync.dma_start(out=wqB[:, 1:4, :], in_=wqT3[:, 1:4, 512:1024])
        nc.sync.dma_start(out=mask_sb[:], in_=maskd)
        for st in range(NT):
            sl = slice(st * 128, (st + 1) * 128)
            if st == 0:
                xt = xtF
            else:
                xt = xtp.tile([128, 16, 128], BF16, tag="xt")
                # first three tiles ride the scalar queue (fresh buffers,
                # WAR-free triggers); later ones would head-of-line block
                # ACT on the buffer-reuse semaphore, so they go on sync
                dmae = nc.scalar if st <= 3 else nc.sync
                dmae.dma_start(out=xt[:], in_=xT3[:, :, sl])
            if st == 0:
                for c in range(1, 4):
                    nc.scalar.dma_start(out=wqA[:, 4 * c:4 * c + 4, :],
                                        in_=wqT3[:, 4 * c:4 * c + 4, 0:512])
                    nc.scalar.dma_start(out=wqB[:, 4 * c:4 * c + 4, :],
                                        in_=wqT3[:, 4 * c:4 * c + 4, 512:1024])
            elif st == 2:
                nc.sync.dma_start(out=wkH[:], in_=wkT3[:, 0:4, :])
            ps0 = psSC.tile([128, 4, 128], F32, tag="sc")
            ps1 = psSC.tile([128, 4, 128], F32, tag="sc")
            for hs in range(16):
                nc.tensor.matmul(ps0[:], lhsT=xt[:, hs], rhs=wqA[:, hs],
                                 start=(hs == 0), stop=(hs == 15))
                nc.tensor.matmul(ps1[:], lhsT=xt[:, hs], rhs=wqB[:, hs],
                                 start=(hs == 0), stop=(hs == 15))
            ct = trig.tile([128, 128], BF16, tag="trig")
            stt = trig.tile([128, 128], BF16, tag="trig")
            nc.sync.dma_start(out=ct[:], in_=cosq[sl, :])
            nc.sync.dma_start(out=stt[:], in_=sinq[sl, :])
            qsb = qsbp.tile([128, 1024], BF16, tag="qsb")
            rmsi0 = process_qk(ps0, ct, stt, qsb[:, 0:512])
            rmsi1 = process_qk(ps1, ct, stt, qsb[:, 512:1024])
            pend_q.append((qsb, rmsi0, rmsi1, sl))
            if len(pend_q) > 1:
                pqsb, pr0, pr1, psl = pend_q.pop(0)
                transpose4(pqsb[:, 0:512], pr0, QT[:, 0:4, psl])
                transpose4(pqsb[:, 512:1024], pr1, QT[:, 4:8, psl])

        # ---- sweep 2: K/V projections; K rms/rope + transpose; V copy ----
        pend_k = []
        wkS = wsl.tile([128, 16, 512], BF16, tag="w")
        wvS = wsl.tile([128, 16, 512], BF16, tag="w")
        for c in range(1, 4):
            nc.sync.dma_start(out=wkS[:, 4 * c:4 * c + 4, :],
                              in_=wkT3[:, 4 * c:4 * c + 4, :])
        for c in range(4):
            nc.sync.dma_start(out=wvS[:, 4 * c:4 * c + 4, :],
                              in_=wvT3[:, 4 * c:4 * c + 4, :])
        for st in range(NT):
            sl = slice(st * 128, (st + 1) * 128)
            if st == 0:
                xt = xtF
            else:
                xt = xtp.tile([128, 16, 128], BF16, tag="xt")
                dmae = nc.scalar if st <= 3 else nc.sync
                dmae.dma_start(out=xt[:], in_=xT3[:, :, sl])
            psk = psSC.tile([128, 4, 128], F32, tag="sc")
            psv = psSC.tile([128, 4, 128], F32, tag="sc")
            for hs in range(16):
                wk_rhs = wkH[:, hs] if hs < 4 else wkS[:, hs]
                nc.tensor.matmul(psk[:], lhsT=xt[:, hs], rhs=wk_rhs,
                                 start=(hs == 0), stop=(hs == 15))
            for hs in range(16):
                nc.tensor.matmul(psv[:], lhsT=xt[:, hs], rhs=wvS[:, hs],
                                 start=(hs == 0), stop=(hs == 15))
            if st == 0:
                # sweep-1 leftover Q transposes: after the K/V matmuls so
                # the last tile's rope chain latency hides under PE work
                for pqsb, pr0, pr1, psl in pend_q:
                    transpose4(pqsb[:, 0:512], pr0, QT[:, 0:4, psl])
                    transpose4(pqsb[:, 512:1024], pr1, QT[:, 4:8, psl])
                pend_q = []
            ct = trig.tile([128, 128], BF16, tag="trig")
            stt = trig.tile([128, 128], BF16, tag="trig")
            nc.sync.dma_start(out=ct[:], in_=cosk[sl, :])
            nc.sync.dma_start(out=stt[:], in_=sink[sl, :])
            ksb = ksbp.tile([128, 512], BF16, tag="ksb")
            rmsik = process_qk(psk, ct, stt, ksb[:, 0:512])
            nc.scalar.copy(Vs[:, st, :], psv[:])
            pend_k.append((ksb, rmsik, sl))
            if len(pend_k) > 1:
                pksb, prk, psl = pend_k.pop(0)
                transpose4(pksb[:, 0:512], prk, KT[:, 0:4, psl])
            if 6 <= st <= 13:
                # weave attention q-chunk 0 (one head per s-tile) into the
                # PE-dense sweep: its KT/Vs tiles 0..3 are long since ready
                # and sweep-2's ACT slack absorbs the exp stream for free
                h0 = st - 6
                pso0, pacc0 = attn_head(0, h0, lambda: None)
                head_epilogue(0, h0, pso0, pacc0)

        for pksb, prk, psl in pend_k:
            transpose4(pksb[:, 0:512], prk, KT[:, 0:4, psl])
        pend_k = []

        # wo weights: the wsl buffers free when the last K/V projection
        # reads them; the DMA triggers sit on the (idle) sync queue
        woA = wsl.tile([128, 4, 2048], BF16, tag="w")
        woB = wsl.tile([128, 4, 2048], BF16, tag="w")
        nc.sync.dma_start(out=woA[:], in_=woT3[:, 0:4, :])
        nc.sync.dma_start(out=woB[:], in_=woT3[:, 4:8, :])

        # attention + wo tail: qc=0 was woven into sweep 2 above; the
        # remaining q-chunk order 2,3,1 gives the long chunks (many
        # k-tiles, thin PE margin over ACT exp) the richest wo filler
        # ratio, and the pure-PE tail is qc=1's wo units.
        def qc_units(qc):
            return [(st, oc) for st in range(4 * qc, 4 * qc + 4)
                    for oc in range(4)]

        attention_block(2, qc_units(0), delay_kts=24)
        attention_block(3, qc_units(2))
        attention_block(1, qc_units(3))
        for _ in wo_feeder(qc_units(1)):
            pass


def build_program():
    nc = bacc.Bacc("TRN2", target_bir_lowering=False, debug=False,
                   num_devices=8)
    shapes = {
        "xT": ([HSD, S], BF16), "wqT": ([HSD, NQ * D], BF16),
        "wkT": ([HSD, NKV * D], BF16), "wvT": ([HSD, NKV * D], BF16),
        "woT": ([NQ * D, HSD], BF16),
        "cosq": ([S, D], BF16), "sinq": ([S, D], BF16),
        "cosk": ([S, D], BF16), "sink": ([S, D], BF16),
        "mask": ([128, 128], BF16),
    }
    aps = {n: nc.dram_tensor(n, sh, dt, kind="ExternalInput").ap()
           for n, (sh, dt) in shapes.items()}
    aps["out"] = nc.dram_tensor("out", [S, HSD], F32,
                                kind="ExternalOutput").ap()

    with tile.TileContext(nc) as tc:
        _emit(tc, aps)
    nc.compile()
    return nc


def make_in_maps(x, cos, sin, wq, wk, wv, wo, q_norm_w, k_norm_w):
    """Host-side preprocessing + sharding into 8 per-core input maps."""
    sign = np.where(np.arange(D) < 64, -1.0, 1.0).astype(np.float32)
    wrot_q = q_norm_w[(np.arange(D) + 64) % D]
    wrot_k = k_norm_w[(np.arange(D) + 64) % D]
    rsd = 1.0 / np.sqrt(np.float32(D))
    cos_q = (cos * q_norm_w[None, :] * rsd).astype(np.float32)
    sin_q = (sin * sign[None, :] * wrot_q[None, :] * rsd).astype(np.float32)
    cos_k = (cos * k_norm_w[None, :]).astype(np.float32)
    sin_k = (sin * sign[None, :] * wrot_k[None, :]).astype(np.float32)
    ii = np.arange(128)
    mask = np.where(ii[None, :] >= ii[:, None], 1.0, 0.0).astype(np.float32)

    def bf(a):
        return np.ascontiguousarray(a).astype(BF)

    in_maps = []
    for ci in range(8):
        b, g = ci // 2, ci % 2
        in_maps.append({
            "xT": bf(x[b].T),
            "wqT": bf(wq[g * 1024:(g + 1) * 1024, :].T),
            "wkT": bf(wk[g * 512:(g + 1) * 512, :].T),
            "wvT": bf(wv[g * 512:(g + 1) * 512, :].T),
            "woT": bf(wo[:, g * 1024:(g + 1) * 1024].T),
            "cosq": bf(cos_q), "sinq": bf(sin_q),
            "cosk": bf(cos_k), "sink": bf(sin_k),
            "mask": bf(mask),
        })
    return in_maps


_NC_CACHE = {}


def kernel(x, cos, sin, wq, wk, wv, wo, q_norm_w, k_norm_w, _results=None,
           **run_kwargs):
    x = np.asarray(x, np.float32)
    in_maps = make_in_maps(np.asarray(x, np.float32), np.asarray(cos, np.float32),
                           np.asarray(sin, np.float32), np.asarray(wq, np.float32),
                           np.asarray(wk, np.float32), np.asarray(wv, np.float32),
                           np.asarray(wo, np.float32),
                           np.asarray(q_norm_w, np.float32),
                           np.asarray(k_norm_w, np.float32))
    if "nc" not in _NC_CACHE:
        _NC_CACHE["nc"] = build_program()
    nc = _NC_CACHE["nc"]
    res = run_bass_kernel_spmd(nc, in_maps, core_ids=list(range(8)),
                               **run_kwargs)
    if _results is not None:
        _results.append(res)
    B = x.shape[0]
    out = np.zeros((B, S, HSD), np.float32)
    for b in range(B):
        out[b] = res.results[2 * b]["out"] + res.results[2 * b + 1]["out"]
    return out


# revision 21
# speedup vs baseline: 1.2128x; 1.2128x over previous
"""Trainium2 Bass kernel for nn_Attention_21285857919576.

Strategy: 8 cores = 4 batches x 2 head-groups (tensor parallel over heads).
Each core computes, for its (batch b, head-group g):
  - Q/K/V projections (s-major) on TensorE in bf16 (lhsT = x^T tiles)
  - per-head sumsq on DVE; rsqrt via ACT exp(-0.5*ln(ms)) (single table set);
    RoPE applied to the UNNORMALIZED q/k (rotation commutes with the
    per-head rms scalar) on DVE + GpSimd
  - transpose+normalize q,k into d-major slabs (QT/KT) via PE matmuls
    against diag(rms_inv) tiles, software-pipelined one s-tile late
  - causal attention in transposed orientation (no transposes of p needed):
      scoresT[k,q] = KT_tile.T @ QT   (one matmul per k-tile, no accum)
      exp on ACT (no max subtraction: RMSNorm+RoPE bound |scores|; 1/sqrt(D)
      and the norm weights are folded into cos/sin host-side)
      p_acc += p on DVE (bf16 2x mode) -- the softmax denominator leaves
      the PE: ONE ones[128,128].T @ p_acc matmul per (head, q-chunk)
      replaces the per-k-tile den matmul stream (saves ~50us of PE issue)
      outT[d,q]  += V_tile.T @ p      (V is s-major straight from projection;
      AV delayed two k-tiles so the exp chain never head-of-line blocks PE)
  - partial output = attnoutT.T @ woT, interleaved INTO the attention phase:
    the wo matmuls of q-chunk qc-1 are slotted between the attention k-tiles
    of q-chunk qc (fractional-credit pacing), so ACT exp hides under wo
    PE streams instead of pacing the PE
Host sums the two partial outputs per batch. No collectives; the 8 cores are
fully independent and perfectly load balanced.

All large matmuls run in bf16 (fp32 PSUM accumulate). Host-side preprocessing
folds q/k norm weights, the rotate-half sign, and the score scale into the
cos/sin tables, and pre-transposes x and the weights so every DMA is
layout-natural. Causal masking is a multiplicative 0/1 bf16 mask applied to p
AFTER exp (exact -- unmasked scores are bounded so exp is finite).
The first x-tile lives in a persistent SBUF tile loaded once at startup and
reused by BOTH projection sweeps (kills the sweep-boundary DMA stall).
PSUM: 4 rotating score banks + 2 attention-out banks + 2 wo banks.
"""
import sys
import numpy as np
import ml_dtypes

for _p in ("/opt/trn_rl_repo", "/opt/pypackages"):
    if _p not in sys.path:
        sys.path.append(_p)

import concourse.bass as bass
from concourse import bacc, mybir, tile
from concourse.bass_utils import run_bass_kernel_spmd


def _install_ntff_hook_shim():
    """The staged antenv package lacks axon_hooks; provide it so
    run_bass_kernel_spmd(trace=True) can drive NTFF profiling via the
    injected libaxon .so (same mechanism trn_boot would register)."""
    import types
    if "antenv.axon_hooks" in sys.modules:
        return
    mod = types.ModuleType("antenv.axon_hooks")
    _state = {"hook": None}
    mod.set_axon_ntff_profile_hook = lambda h: _state.__setitem__("hook", h)
    mod.get_axon_ntff_profile_hook = lambda: _state["hook"]
    sys.modules["antenv.axon_hooks"] = mod
    try:
        import antenv
        antenv.axon_hooks = mod
    except ImportError:
        pass
    try:
        from trn_agent_boot.trn_boot import _ntff_profile_via_ctypes
        import os
        so = "/opt/axon/libaxon_pjrt.so"
        if os.path.exists(so):
            mod.set_axon_ntff_profile_hook(_ntff_profile_via_ctypes(so))
    except Exception:
        pass


_install_ntff_hook_shim()


def _install_act_table_patch():
    """Force Exp/Ln/Copy/Square onto the single natural_log_exp_and_others
    ACT table set: blank every other set containing exp/ln so the
    table-load chooser can't alternate between sets (each switch costs
    ~2.7us and we interleave Ln (rms) with Exp (softmax))."""
    import concourse.hw_specs as hw_specs
    import concourse.bacc as bacc_mod
    if getattr(hw_specs, "_act_patch", False):
        return
    orig = hw_specs.get_activation_tables

    def patched(module_arch):
        tables = orig(module_arch)
        keep = "natural_log_exp_and_others"
        if keep in tables:
            for name, fns in tables.items():
                if name != keep and any(f.name in ("Exp", "Ln") for f in fns):
                    tables[name] = set()
        return tables

    hw_specs.get_activation_tables = patched
    if getattr(bacc_mod, "get_activation_tables", None) is orig:
        bacc_mod.get_activation_tables = patched
    hw_specs._act_patch = True


_install_act_table_patch()

BF = ml_dtypes.bfloat16
F32 = mybir.dt.float32
BF16 = mybir.dt.bfloat16
ALU = mybir.AluOpType
AF = mybir.ActivationFunctionType

S, HSD, D = 2048, 2048, 128
NQ, NKV = 8, 4          # per-core q heads / kv heads
NT = S // 128           # 16 s-tiles
EPS = 1e-6


def _emit(tc, aps):
    from collections import deque
    nc = tc.nc
    xT = aps["xT"]
    wqT = aps["wqT"]
    wkT = aps["wkT"]
    wvT = aps["wvT"]
    woT = aps["woT"]
    cosq = aps["cosq"]
    sinq = aps["sinq"]
    cosk = aps["cosk"]
    sink = aps["sink"]
    maskd = aps["mask"]
    outd = aps["out"]

    xT3 = xT.rearrange("(ho hi) s -> hi ho s", hi=128)       # [128, 16, 2048]
    wqT3 = wqT.rearrange("(ho hi) f -> hi ho f", hi=128)     # [128, 16, 1024]
    wkT3 = wkT.rearrange("(ho hi) f -> hi ho f", hi=128)     # [128, 16, 512]
    wvT3 = wvT.rearrange("(ho hi) f -> hi ho f", hi=128)
    woT3 = woT.rearrange("(fo fi) o -> fi fo o", fi=128)     # [128, 8, 2048]

    from contextlib import ExitStack
    with ExitStack() as ctx:
        singles = ctx.enter_context(tc.tile_pool(name="singles", bufs=1))
        wsl = ctx.enter_context(tc.tile_pool(name="wsl", bufs=2))
        xtp = ctx.enter_context(tc.tile_pool(name="xtp", bufs=3))
        trig = ctx.enter_context(tc.tile_pool(name="trig", bufs=4))
        qfp = ctx.enter_context(tc.tile_pool(name="qfp", bufs=4))
        t12 = ctx.enter_context(tc.tile_pool(name="t12", bufs=6))
        sqp = ctx.enter_context(tc.tile_pool(name="sqp", bufs=2))
        tiny = ctx.enter_context(tc.tile_pool(name="tiny", bufs=8))
        qsbp = ctx.enter_context(tc.tile_pool(name="qsbp", bufs=3))
        ksbp = ctx.enter_context(tc.tile_pool(name="ksbp", bufs=3))
        pp = ctx.enter_context(tc.tile_pool(name="pp", bufs=8))
        outp = ctx.enter_context(tc.tile_pool(name="outp", bufs=3))
        rdenp = ctx.enter_context(tc.tile_pool(name="rdenp", bufs=3))
        diagp = ctx.enter_context(tc.tile_pool(name="diagp", bufs=4))
        paccp = ctx.enter_context(tc.tile_pool(name="paccp", bufs=2))
        psSC = ctx.enter_context(tc.tile_pool(name="psSC", bufs=4, space="PSUM"))
        psoP = ctx.enter_context(tc.tile_pool(name="psoP", bufs=2,
                                              space="PSUM"))
        woPp = ctx.enter_context(tc.tile_pool(name="woPp", bufs=2,
                                              space="PSUM"))

        QT = singles.tile([128, NQ, S], BF16)    # [d, h, s]
        KT = singles.tile([128, NKV, S], BF16)   # [d, kv, s]
        Vs = singles.tile([128, NT, NKV * D], BF16)  # [s_i, s_o, f]
        AOT = singles.tile([128, NQ, S], BF16)   # [d, h, s]
        mask_sb = singles.tile([128, 128], BF16)
        ones_col = singles.tile([128, 128], BF16)

        eps_sb = singles.tile([128, 1], F32)
        wkH = singles.tile([128, 4, 512], BF16)
        ident = singles.tile([128, 128], BF16)
        xtF = singles.tile([128, 16, 128], BF16)   # first s-tile, both sweeps
        nc.vector.memset(ones_col[:], 1.0)
        nc.vector.memset(eps_sb[:], EPS)
        from concourse.masks import make_identity
        make_identity(nc, ident[:])

        def process_qk(ps, cos_t, sin_t, out_ap):
            """RoPE 4 heads from psum tile ps [128,4,128] (unnormalized),
            write bf16 to out_ap [128, 512]; return rms_inv [128, 4] tile
            (applied later via the diag transpose matmul)."""
            qf = qfp.tile([128, 4, 128], BF16, tag="qf")
            nc.scalar.copy(qf[:], ps[:])          # ACT: psum -> sbuf
            sq = sqp.tile([128, 4, 128], BF16, tag="sq")
            ss = tiny.tile([128, 4], F32, tag="ss")
            for hb in range(4):
                nc.vector.scalar_tensor_tensor(
                    out=sq[:, hb], in0=qf[:, hb], scalar=1.0, in1=qf[:, hb],
                    op0=ALU.mult, op1=ALU.mult, accum_out=ss[:, hb:hb + 1])
            lnt = tiny.tile([128, 4], F32, tag="lnt")
            nc.scalar.activation(lnt[:], ss[:], AF.Ln, scale=1.0 / D,
                                 bias=eps_sb[:])
            rmsi = tiny.tile([128, 4], BF16, tag="rmsi")
            nc.scalar.activation(rmsi[:], lnt[:], AF.Exp, scale=-0.5)
            cosb = cos_t[:].unsqueeze(1).to_broadcast([128, 4, 128])
            sinb = sin_t[:].unsqueeze(1).to_broadcast([128, 4, 128])
            t1 = t12.tile([128, 4, 128], BF16, tag="t1")
            t2 = t12.tile([128, 4, 128], BF16, tag="t2")
            nc.vector.tensor_tensor(out=t1[:], in0=qf[:], in1=cosb, op=ALU.mult)
            qfb = qf[:]
            rot = bass.AP(tensor=qfb.tensor, offset=qfb.offset + 64,
                          ap=[qfb.ap[0], [128, 4], [-64, 2], [1, 64]])
            t2v = t2[:].rearrange("p h (j d) -> p h j d", j=2)
            sinv = sinb.rearrange("p h (j d) -> p h j d", j=2)
            nc.vector.tensor_tensor(out=t2v, in0=rot, in1=sinv, op=ALU.mult)
            nc.gpsimd.tensor_tensor(out=out_ap, in0=t1[:], in1=t2[:], op=ALU.add)
            return rmsi

        def transpose4(src_ap, rmsi, dst_ap):
            """Transpose+normalize 4 head-tiles: src_ap [128 s, 512 f] bf16
            (roped, unnormalized), rmsi [128 s, 4], into dst_ap
            [128 d, 4, 128 s] (bf16 slab slice) via PE diag matmuls."""
            pst = psSC.tile([128, 4, 128], F32, tag="sc")
            diag4 = diagp.tile([128, 4, 128], BF16, tag="diag")
            identb = ident[:].unsqueeze(1).to_broadcast([128, 4, 128])
            rmsib = rmsi[:].unsqueeze(2).to_broadcast([128, 4, 128])
            nc.vector.tensor_tensor(out=diag4[:], in0=identb, in1=rmsib,
                                    op=ALU.mult)
            for hb in range(4):
                nc.tensor.matmul(pst[:, hb],
                                 lhsT=src_ap[:, hb * 128:(hb + 1) * 128],
                                 rhs=diag4[:, hb], start=True, stop=True)
            nc.scalar.copy(dst_ap, pst[:])

        def attn_head(qc, h, feed_credit):
            """QK -> exp -> p_acc (DVE) -> AV for one (q-chunk, head).
            AV delayed two k-tiles; den is a single matmul off p_acc.
            feed_credit() is called once per k-tile to slot wo-projection
            matmuls into the PE stream so ACT exp time stays hidden."""
            qbase = qc * 512
            kv = h // 2
            nkt = 4 * qc + 4
            pso = psoP.tile([128, 512], F32, tag="pso")
            pacc = paccp.tile([128, 512], BF16, tag="pacc")

            def emit_av(p, kt, lo):
                nc.tensor.matmul(
                    pso[:, lo:512],
                    lhsT=Vs[:, kt, kv * 128:(kv + 1) * 128],
                    rhs=p[:, lo:512],
                    start=(kt == 0), stop=(kt == nkt - 1))

            pend_av = deque()
            for kt in range(nkt):
                j = kt - 4 * qc
                lo = 128 * j if j >= 0 else 0
                psc = psSC.tile([128, 512], F32, tag="sc")
                nc.tensor.matmul(
                    psc[:, lo:512],
                    lhsT=KT[:, kv, kt * 128:(kt + 1) * 128],
                    rhs=QT[:, h, qbase + lo:qbase + 512],
                    start=True, stop=True)
                p = pp.tile([128, 512], BF16, tag="p")
                nc.scalar.activation(p[:, lo:512], psc[:, lo:512], AF.Exp)
                if j >= 0:
                    nc.vector.tensor_tensor(
                        out=p[:, lo:lo + 128], in0=p[:, lo:lo + 128],
                        in1=mask_sb[:], op=ALU.mult)
                if kt == 0:
                    nc.vector.tensor_copy(out=pacc[:], in_=p[:])
                else:
                    nc.vector.tensor_tensor(
                        out=pacc[:, lo:512], in0=pacc[:, lo:512],
                        in1=p[:, lo:512], op=ALU.add)
                pend_av.append((p, kt, lo))
                if len(pend_av) > 3:
                    emit_av(*pend_av.popleft())
                feed_credit()
            while pend_av:
                emit_av(*pend_av.popleft())
            return pso, pacc

        def head_epilogue(qc, h, pso, pacc):
            psden = psSC.tile([128, 512], F32, tag="sc")
            nc.tensor.matmul(psden[:], lhsT=ones_col[:], rhs=pacc[:],
                             start=True, stop=True)
            rden = rdenp.tile([128, 512], F32, tag="rden")
            nc.vector.reciprocal_approx_fast(out=rden[:], in_=psden[:])
            nc.vector.tensor_tensor(
                out=AOT[:, h, qc * 512:qc * 512 + 512], in0=pso[:],
                in1=rden[:], op=ALU.mult)

        def wo_feeder(units):
            """Generator emitting one wo matmul per next(); finishes each
            (s-tile, out-chunk) unit with the psum->sbuf copy + store."""
            for st, oc in units:
                ssl = slice(st * 128, (st + 1) * 128)
                osl = slice(oc * 512, (oc + 1) * 512)
                po = woPp.tile([128, 512], F32, tag="po")
                for fc in range(NQ):
                    w = woA if fc < 4 else woB
                    nc.tensor.matmul(po[:], lhsT=AOT[:, fc, ssl],
                                     rhs=w[:, fc % 4, osl],
                                     start=(fc == 0), stop=(fc == 7))
                    if fc == 7:
                        osb = outp.tile([128, 512], F32, tag="osb")
                        nc.vector.tensor_copy(out=osb[:], in_=po[:])
                        nc.sync.dma_start(out=outd[ssl, osl], in_=osb[:])
                    yield

        def attention_block(qc, units, delay_kts=0):
            """All 8 heads of q-chunk qc; the wo matmuls of q-chunk qc-1
            are slotted between attention k-tiles (via feed_credit) so
            the PE always has filler while ACT exp runs. delay_kts skips
            feeding for the first k-tiles (covers the wo-weight DMA)."""
            feeder = wo_feeder(units)
            nkts = NQ * (4 * qc + 4)
            per_kt = max(0.0, (len(units) * NQ - NQ) / max(1, nkts - delay_kts))
            state = {"c": 0.0, "kt": 0}

            def feed(n):
                for _ in range(n):
                    if next(feeder, None) is None:
                        return

            def feed_credit():
                state["kt"] += 1
                if state["kt"] <= delay_kts:
                    return
                state["c"] += per_kt
                while state["c"] >= 1.0:
                    feed(1)
                    state["c"] -= 1.0

            for h in range(NQ):
                pso, pacc = attn_head(qc, h, feed_credit)
                if state["kt"] > delay_kts:
                    feed(1)
                head_epilogue(qc, h, pso, pacc)
            for _ in feeder:
                pass

        # ---- sweep 1: Q projection + rms/rope + transpose into QT ----
        pend_q = []
        wqA = wsl.tile([128, 16, 512], BF16, tag="w")
        wqB = wsl.tile([128, 16, 512], BF16, tag="w")
        nc.scalar.dma_start(out=xtF[:, 0:8, :], in_=xT3[:, 0:8, 0:128])
        nc.sync.dma_start(out=wqA[:, 0:1, :], in_=wqT3[:, 0:1, 0:512])
        nc.sync.dma_start(out=wqB[:, 0:1, :], in_=wqT3[:, 0:1, 512:1024])
        nc.scalar.dma_start(out=xtF[:, 8:16, :], in_=xT3[:, 8:16, 0:128])
        nc.sync.dma_start(out=wqA[:, 1:4, :], in_=wqT3[:, 1:4, 0:512])
        nc.sync.dma_start(out=wqB[:, 1:4, :], in_=wqT3[:, 1:4, 512:1024])
        nc.sync.dma_start(out=mask_sb[:], in_=maskd)
        for st in range(NT):
            sl = slice(st * 128, (st + 1) * 128)
            if st == 0:
                xt = xtF
            else:
                xt = xtp.tile([128, 16, 128], BF16, tag="xt")
                # first three tiles ride the scalar queue (fresh buffers,
                # WAR-free triggers); later ones would head-of-line block
                # ACT on the buffer-reuse semaphore, so they go on sync
                dmae = nc.scalar if st <= 3 else nc.sync
                dmae.dma_start(out=xt[:], in_=xT3[:, :, sl])
            if st == 0:
                for c in range(1, 4):
                    nc.scalar.dma_start(out=wqA[:, 4 * c:4 * c + 4, :],
                                        in_=wqT3[:, 4 * c:4 * c + 4, 0:512])
                    nc.scalar.dma_start(out=wqB[:, 4 * c:4 * c + 4, :],
                                        in_=wqT3[:, 4 * c:4 * c + 4, 512:1024])
            elif st == 2:
                nc.sync.dma_start(out=wkH[:], in_=wkT3[:, 0:4, :])
            ps0 = psSC.tile([128, 4, 128], F32, tag="sc")
            ps1 = psSC.tile([128, 4, 128], F32, tag="sc")
            for hs in range(16):
                nc.tensor.matmul(ps0[:], lhsT=xt[:, hs], rhs=wqA[:, hs],
                                 start=(hs == 0), stop=(hs == 15))
                nc.tensor.matmul(ps1[:], lhsT=xt[:, hs], rhs=wqB[:, hs],
                                 start=(hs == 0), stop=(hs == 15))
            ct = trig.tile([128, 128], BF16, tag="trig")
            stt = trig.tile([128, 128], BF16, tag="trig")
            nc.sync.dma_start(out=ct[:], in_=cosq[sl, :])
            nc.sync.dma_start(out=stt[:], in_=sinq[sl, :])
            qsb = qsbp.tile([128, 1024], BF16, tag="qsb")
            rmsi0 = process_qk(ps0, ct, stt, qsb[:, 0:512])
            rmsi1 = process_qk(ps1, ct, stt, qsb[:, 512:1024])
            pend_q.append((qsb, rmsi0, rmsi1, sl))
            if len(pend_q) > 1:
                pqsb, pr0, pr1, psl = pend_q.pop(0)
                transpose4(pqsb[:, 0:512], pr0, QT[:, 0:4, psl])
                transpose4(pqsb[:, 512:1024], pr1, QT[:, 4:8, psl])

        # ---- sweep 2: K/V projections; K rms/rope + transpose; V copy ----
        pend_k = []
        wkS = wsl.tile([128, 16, 512], BF16, tag="w")
        wvS = wsl.tile([128, 16, 512], BF16, tag="w")
        for c in range(1, 4):
            nc.sync.dma_start(out=wkS[:, 4 * c:4 * c + 4, :],
                              in_=wkT3[:, 4 * c:4 * c + 4, :])
        for c in range(4):
            nc.sync.dma_start(out=wvS[:, 4 * c:4 * c + 4, :],
                              in_=wvT3[:, 4 * c:4 * c + 4, :])
        for st in range(NT):
            sl = slice(st * 128, (st + 1) * 128)
            if st == 0:
                xt = xtF
            else:
                xt = xtp.tile([128, 16, 128], BF16, tag="xt")
                dmae = nc.scalar if st <= 3 else nc.sync
                dmae.dma_start(out=xt[:], in_=xT3[:, :, sl])
            psk = psSC.tile([128, 4, 128], F32, tag="sc")
            psv = psSC.tile([128, 4, 128], F32, tag="sc")
            for hs in range(16):
                wk_rhs = wkH[:, hs] if hs < 4 else wkS[:, hs]
                nc.tensor.matmul(psk[:], lhsT=xt[:, hs], rhs=wk_rhs,
                                 start=(hs == 0), stop=(hs == 15))
            for hs in range(16):
                nc.tensor.matmul(psv[:], lhsT=xt[:, hs], rhs=wvS[:, hs],
                                 start=(hs == 0), stop=(hs == 15))
            if st == 0:
                # sweep-1 leftover Q transposes: after the K/V matmuls so
                # the last tile's rope chain latency hides under PE work
                for pqsb, pr0, pr1, psl in pend_q:
                    transpose4(pqsb[:, 0:512], pr0, QT[:, 0:4, psl])
                    transpose4(pqsb[:, 512:1024], pr1, QT[:, 4:8, psl])
                pend_q = []
            ct = trig.tile([128, 128], BF16, tag="trig")
            stt = trig.tile([128, 128], BF16, tag="trig")
            nc.sync.dma_start(out=ct[:], in_=cosk[sl, :])
            nc.sync.dma_start(out=stt[:], in_=sink[sl, :])
            ksb = ksbp.tile([128, 512], BF16, tag="ksb")
            rmsik = process_qk(psk, ct, stt, ksb[:, 0:512])
            nc.scalar.copy(Vs[:, st, :], psv[:])
            pend_k.append((ksb, rmsik, sl))
            if len(pend_k) > 1:
                pksb, prk, psl = pend_k.pop(0)
                transpose4(pksb[:, 0:512], prk, KT[:, 0:4, psl])

        for pksb, prk, psl in pend_k:
            transpose4(pksb[:, 0:512], prk, KT[:, 0:4, psl])
        pend_k = []

        # wo weights: the wsl buffers free when the last K/V projection
        # reads them; the DMA triggers sit on the (idle) sync queue
        woA = wsl.tile([128, 4, 2048], BF16, tag="w")
        woB = wsl.tile([128, 4, 2048], BF16, tag="w")
        nc.sync.dma_start(out=woA[:], in_=woT3[:, 0:4, :])
        nc.sync.dma_start(out=woB[:], in_=woT3[:, 4:8, :])

        # attention + wo tail: q-chunk order 0,2,3,1 — qc=0 (cheapest) runs
        # unfilled while the wo weights load; the long chunks get the
        # richest wo filler ratio; the pure-PE tail is qc=1's wo units.
        def qc_units(qc):
            return [(st, oc) for st in range(4 * qc, 4 * qc + 4)
                    for oc in range(4)]

        attention_block(0, [])
        attention_block(2, qc_units(0))
        attention_block(3, qc_units(2))
        attention_block(1, qc_units(3))
        for _ in wo_feeder(qc_units(1)):
            pass


def build_program():
    nc = bacc.Bacc("TRN2", target_bir_lowering=False, debug=False,
                   num_devices=8)
    shapes = {
        "xT": ([HSD, S], BF16), "wqT": ([HSD, NQ * D], BF16),
        "wkT": ([HSD, NKV * D], BF16), "wvT": ([HSD, NKV * D], BF16),
        "woT": ([NQ * D, HSD], BF16),
        "cosq": ([S, D], BF16), "sinq": ([S, D], BF16),
        "cosk": ([S, D], BF16), "sink": ([S, D], BF16),
        "mask": ([128, 128], BF16),
    }
    aps = {n: nc.dram_tensor(n, sh, dt, kind="ExternalInput").ap()
           for n, (sh, dt) in shapes.items()}
    aps["out"] = nc.dram_tensor("out", [S, HSD], F32,
                                kind="ExternalOutput").ap()

    with tile.TileContext(nc) as tc:
        _emit(tc, aps)
    nc.compile()
    return nc


def make_in_maps(x, cos, sin, wq, wk, wv, wo, q_norm_w, k_norm_w):
    """Host-side preprocessing + sharding into 8 per-core input maps."""
    sign = np.where(np.arange(D) < 64, -1.0, 1.0).astype(np.float32)
    wrot_q = q_norm_w[(np.arange(D) + 64) % D]
    wrot_k = k_norm_w[(np.arange(D) + 64) % D]
    rsd = 1.0 / np.sqrt(np.float32(D))
    cos_q = (cos * q_norm_w[None, :] * rsd).astype(np.float32)
    sin_q = (sin * sign[None, :] * wrot_q[None, :] * rsd).astype(np.float32)
    cos_k = (cos * k_norm_w[None, :]).astype(np.float32)
    sin_k = (sin * sign[None, :] * wrot_k[None, :]).astype(np.float32)
    ii = np.arange(128)
    mask = np.where(ii[None, :] >= ii[:, None], 1.0, 0.0).astype(np.float32)

    def bf(a):
        return np.ascontiguousarray(a).astype(BF)

    in_maps = []
    for ci in range(8):
        b, g = ci // 2, ci % 2
        in_maps.append({
            "xT": bf(x[b].T),
            "wqT": bf(wq[g * 1024:(g + 1) * 1024, :].T),
            "wkT": bf(wk[g * 512:(g + 1) * 512, :].T),
            "wvT": bf(wv[g * 512:(g + 1) * 512, :].T),
            "woT": bf(wo[:, g * 1024:(g + 1) * 1024].T),
            "cosq": bf(cos_q), "sinq": bf(sin_q),
            "cosk": bf(cos_k), "sink": bf(sin_k),
            "mask": bf(mask),
        })
    return in_maps


_NC_CACHE = {}


def kernel(x, cos, sin, wq, wk, wv, wo, q_norm_w, k_norm_w, _results=None,
           **run_kwargs):
    x = np.asarray(x, np.float32)
    in_maps = make_in_maps(np.asarray(x, np.float32), np.asarray(cos, np.float32),
                           np.asarray(sin, np.float32), np.asarray(wq, np.float32),
                           np.asarray(wk, np.float32), np.asarray(wv, np.float32),
                           np.asarray(wo, np.float32),
                           np.asarray(q_norm_w, np.float32),
                           np.asarray(k_norm_w, np.float32))
    if "nc" not in _NC_CACHE:
        _NC_CACHE["nc"] = build_program()
    nc = _NC_CACHE["nc"]
    res = run_bass_kernel_spmd(nc, in_maps, core_ids=list(range(8)),
                               **run_kwargs)
    if _results is not None:
        _results.append(res)
    B = x.shape[0]
    out = np.zeros((B, S, HSD), np.float32)
    for b in range(B):
        out[b] = res.results[2 * b]["out"] + res.results[2 * b + 1]["out"]
    return out
